# revision 10
# baseline (speedup 1.0000x reference)
import sys
import numpy as np

for _p in ("/opt/trn_rl_repo",):
    if _p not in sys.path:
        sys.path.insert(0, _p)

import concourse.bass as bass
import concourse.mybir as mybir
import concourse.tile as tile
from concourse.bass_utils import run_bass_kernel_spmd

TEMPERATURE = 0.07
EPS = 1e-08
HARD_NEG_WEIGHT = 2.0
DOMAIN_SEP_WEIGHT = 1.5

B, L, D = 32, 256, 256
N = B * L            # 8192
NCORES = 8
ROWS = N // NCORES   # 1024 rows of sim per core
F32 = mybir.dt.float32

_CACHE = {}


def _build_nc():
    """Per-core program: simblk[1024, 8192] = xblkT.T @ xT (raw dot products)."""
    nc = bass.Bass()
    # xin = [xTfull | xTblk] concatenated along columns: one DMA -> one wait sem
    xin = nc.declare_dram_parameter("xin", [D, N + ROWS], F32, isOutput=False)
    simblk = nc.declare_dram_parameter("simblk", [ROWS, N], F32, isOutput=True)

    KT = D // 128          # 2 k-tiles
    MT = ROWS // 128       # 8 row tiles of 128 rows
    NBW = 512              # psum free dim
    NT = N // NBW          # 16 col tiles per row
    RB = 3                 # row staging buffers
    TILES = MT * NT        # 128 psum tiles total

    with (
        nc.sbuf_tensor([128, KT, N + ROWS], F32) as xf,
        nc.sbuf_tensor([128, RB, N], F32) as rows,
        nc.psum_tensor([128, 8, NBW], F32) as ps,
        nc.semaphore("s_in") as s_in,
        nc.semaphore("s_pe") as s_pe,
        nc.semaphore("s_cp") as s_cp,
        nc.semaphore("s_o0") as s_o0,
        nc.semaphore("s_o1") as s_o1,
        nc.semaphore("s_o2") as s_o2,
        nc.Block() as block,
    ):
        s_out = [s_o0, s_o1, s_o2]

        @block.sync
        def _(sync):
            sync.dma_start(
                xf[:], xin.rearrange("(k p) n -> p k n", p=128)
            ).then_inc(s_in, 16)
            for m in range(MT):
                # all 16 copies of row m staged
                sync.wait_ge(s_cp, 16 * (m + 1))
                sync.dma_start(
                    simblk[m * 128:(m + 1) * 128, :], rows[:, m % RB, :]
                ).then_inc(s_out[m % RB], 16)

        @block.tensor
        def _(tensor):
            tensor.wait_ge(s_in, 16)
            for t in range(TILES):
                m, n = divmod(t, NT)
                b = t % 8
                if t >= 8:
                    # psum bank b free once copy of tile t-8 completed
                    tensor.wait_ge(s_cp, t - 7)
                for k in range(KT):
                    nc.tensor.matmul(
                        ps[:, b, :],
                        xf[:, k, N + m * 128:N + (m + 1) * 128],
                        xf[:, k, n * NBW:(n + 1) * NBW],
                        start=(k == 0),
                        stop=(k == KT - 1),
                    ).then_inc(s_pe, 1)

        @block.vector
        def _(vector):
            for t in range(TILES):
                m, n = divmod(t, NT)
                if n == 0 and m >= RB:
                    # row slot reusable once row m-RB fully DMAed out
                    vector.wait_ge(s_out[m % RB], 16 * (m // RB))
                vector.wait_ge(s_pe, 2 * (t + 1))
                nc.vector.tensor_copy(
                    rows[:, m % RB, n * NBW:(n + 1) * NBW], ps[:, t % 8, :]
                ).then_inc(s_cp, 1)

    return nc


def _get_nc():
    if "nc" not in _CACHE:
        _CACHE["nc"] = _build_nc()
    return _CACHE["nc"]


def _run_device(xT, trace=False):
    nc = _get_nc()
    in_maps = [
        {
            "xin": np.ascontiguousarray(
                np.concatenate([xT, xT[:, c * ROWS:(c + 1) * ROWS]], axis=1)
            ),
        }
        for c in range(NCORES)
    ]
    try:
        res = run_bass_kernel_spmd(nc, in_maps, list(range(NCORES)), trace=trace)
    except ModuleNotFoundError:
        # NTFF profile hook unavailable in this container; run without trace
        res = run_bass_kernel_spmd(nc, in_maps, list(range(NCORES)), trace=False)
    sim = np.concatenate([res.results[c]["simblk"] for c in range(NCORES)], axis=0)
    return sim, res


def kernel(feats, dataset_ids, image_ids, _trace=False, _ret_res=False):
    x = np.asarray(feats, dtype=np.float32).reshape(N, D)
    nrm = np.sqrt(np.sum(x * x, axis=1, keepdims=True, dtype=np.float32)).astype(np.float32)
    x = x / np.maximum(nrm, np.float32(EPS))
    xT = np.ascontiguousarray(x.T)

    sim, res = _run_device(xT, trace=_trace)
    sim = sim / np.float32(TEMPERATURE)

    did = np.asarray(dataset_ids).reshape(-1)
    iid = np.asarray(image_ids).reshape(-1)
    same_img = (did[:, None] == did[None, :]) & (iid[:, None] == iid[None, :])
    eye = np.eye(N, dtype=bool)
    pos_mask = same_img & ~eye
    diff_dataset = did[:, None] != did[None, :]

    sim_exp = np.exp(sim)

    cross = sim[diff_dataset]
    if cross.size > 0:
        thr = np.float32(np.quantile(cross, 0.8))
    else:
        thr = np.float32(0.0)
    hard_neg_mask = diff_dataset & (sim > thr)

    neg_weights = np.where(diff_dataset, np.float32(DOMAIN_SEP_WEIGHT), np.float32(1.0)) * \
        np.where(hard_neg_mask, np.float32(HARD_NEG_WEIGHT), np.float32(1.0))

    pos_sum = np.sum(sim_exp * pos_mask.astype(np.float32), axis=1, dtype=np.float32)
    neg_sum = np.sum(sim_exp * neg_weights * (~pos_mask).astype(np.float32), axis=1, dtype=np.float32)

    loss = -np.log((pos_sum + np.float32(EPS)) / (pos_sum + neg_sum + np.float32(EPS)))

    valid = pos_mask.any(axis=1)
    n_valid = valid.sum()
    if n_valid > 0:
        out = np.float32(np.sum(loss * valid.astype(np.float32)) / np.float32(max(n_valid, 1)))
    else:
        out = np.float32(loss.mean())
    out = np.asarray(out, dtype=np.float32)
    if _ret_res:
        return out, res
    return out
